# revision 55
# baseline (speedup 1.0000x reference)
"""GCNConv layer on 8 Trainium2 NeuronCores (Bass/Tile).

out = relu( D^-1/2 (A+I) D^-1/2 x W + b ) + x   (aggregate-then-matmul)

Per core (SPMD, core owns 12500 destination rows):
  - 16 phases = 4 source chunks (25000 rows each, int16 gather indices) x
    4 destination quarters (3125 dsts).  Per phase, dsts are ordered by
    in-degree from that chunk (ELL): the k-th incoming edge of every dst
    forms a prefix of the ordering.  Consecutive passes are fused into
    dma_gathers of up to 4096 rows spread round-robin over the 4 SWDGE
    queues (each queue's descriptor generation runs on its own Q7 pair).
  - each gathered buffer is scaled by dinv_src (DVE, bf16 out) and
    accumulated into PSUM by the PE (bf16 identity matmuls, 1 cycle/col) —
    pass k's slots are a prefix so PSUM regions nest; per-bank start/stop.
  - PSUM banks are drained by the Scalar engine into an SBUF staging tile
    and written densely (partition-major, 128 big descriptors) to a
    per-chunk HBM table; a zero row serves degree-0 (dst,chunk) pairs.
  - merge: per 16-block group, one dma_gather per chunk pulls the four
    permuted contributions back (emitted as soon as the spanned quarters'
    accd writes land; the DVE adds two phases later so the in-order DVE
    stream never blocks); sums land in an HBM agd table.
  - tail epilogue per 4-block group: reload agd + x, scale by dinv_dst,
    PE transpose -> W matmul (bf16) -> fused bias+relu (ACT) -> transpose
    back -> residual add -> store.
  - x shard / output use a partition-major (raster) layout in HBM; the
    host ships xsh pre-swizzled and un-swizzles the output (layout-only).

Edge sets are padded with weight-0 fake edges so all 8 cores run the exact
same static program (SPMD) with per-core data only.
"""

import sys
import types

sys.path.insert(0, "/opt/trn_rl_repo")

import numpy as np

DIM = 64
N_CORES = 8
N_CHUNKS = 4       # source chunks (int16 gather index limit)
N_QUART = 4        # destination quarters (PSUM accumulator capacity)
N_QUEUES = 4
P = 128
FUSE_CAP = 4096    # max slots per fused edge gather
MG_BLK = 16        # dst blocks per merge gather group
EP_BLK = 4         # dst blocks per epilogue group
BANK_F32 = 512     # psum bank = 512 fp32 per partition


class Plan:
    def __init__(self, n_nodes):
        self.N = n_nodes
        self.SHARD = n_nodes // N_CORES          # 12500
        self.CH = n_nodes // N_CHUNKS            # 25000
        assert self.CH <= 32767
        self.QH = self.SHARD // N_QUART          # 3125
        assert self.SHARD % N_QUART == 0
        self.ACCB = -(-self.QH // P)             # 25 blocks per quarter acc
        self.ACC_SLOTS = self.ACCB * P           # 3200
        self.ACC_COLS = self.ACCB * DIM          # 1600 f32
        self.NBANKS = -(-self.ACC_COLS // BANK_F32)   # 4
        self.JJ = N_QUART * self.ACCB + 1        # accd col-blocks/partition (+zero)
        self.SHB = -(-self.SHARD // P)           # 98
        self.n_phases = N_CHUNKS * N_QUART       # 16 (chunk-major order)
        self.n_mg = -(-self.SHB // MG_BLK)       # 7
        self.mg_sizes = [min(MG_BLK, self.SHB - m * MG_BLK) * P
                         for m in range(self.n_mg)]
        self.n_ep = -(-self.SHB // EP_BLK)       # 25
        self.ep_sizes = [min(EP_BLK, self.SHB - e * EP_BLK)
                         for e in range(self.n_ep)]
        self.pass_sizes = None   # [phase] -> padded pass sizes (slots)
        self.fuse = None         # [phase] -> list of (k0, k1, slots)
        self.g16_off = None      # [phase][k] -> gidx col offset (int16 cols)
        self.g128_off = None     # [phase][k] -> gwgt col offset
        self.GCOLS = 0
        self.WCOLS = 0
        # merge group -> set of quarters it spans (for deps)
        self.mg_quarters = []
        for m in range(self.n_mg):
            d0 = m * MG_BLK * P
            d1 = min(d0 + self.mg_sizes[m], self.SHARD) - 1
            self.mg_quarters.append(
                sorted(set([d0 // self.QH, max(d1, d0) // self.QH]))
            )

    def mg_off(self, m, c):
        o = 0
        for mm in range(m):
            o += (self.mg_sizes[mm] // 16) * N_CHUNKS
        return o + (self.mg_sizes[m] // 16) * c


def _rep16(vals_i16, n):
    a = np.asarray(vals_i16, dtype=np.int16).reshape(n // 16, 16).T
    return np.tile(a, (8, 1))


def preprocess(x, edge_index, W, b):
    x = np.ascontiguousarray(np.asarray(x, dtype=np.float32))
    N = x.shape[0]
    plan = Plan(N)
    src = np.asarray(edge_index[0], dtype=np.int64)
    dst = np.asarray(edge_index[1], dtype=np.int64)
    deg = np.bincount(dst, minlength=N).astype(np.float64) + 1.0
    dinv = (1.0 / np.sqrt(deg)).astype(np.float32)

    SHARD, CH, QH = plan.SHARD, plan.CH, plan.QH
    NPH = plan.n_phases

    core_of = dst // SHARD
    per_core = []
    for i in range(N_CORES):
        m = core_of == i
        s_i = src[m]
        d_i = dst[m] - i * SHARD
        c_i = s_i // CH
        q_i = d_i // QH
        phases = []
        for c in range(N_CHUNKS):
            for q in range(N_QUART):
                mm = (c_i == c) & (q_i == q)
                s = s_i[mm]
                d = d_i[mm] - q * QH
                deg_cq = np.bincount(d, minlength=QH)
                order = np.argsort(-deg_cq, kind="stable")
                rank = np.empty(QH, dtype=np.int64)
                rank[order] = np.arange(QH)
                perm = np.argsort(rank[d], kind="stable")
                s_sorted = s[perm]
                counts = deg_cq[order]
                cum = np.concatenate([[0], np.cumsum(counts)])
                K = int(counts[0]) if len(s) else 0
                passes = []
                for k in range(K):
                    L = int(np.searchsorted(-counts, -k, side="left"))
                    passes.append(s_sorted[cum[:L] + k])
                phases.append({"passes": passes, "rank": rank, "deg": deg_cq})
        per_core.append(phases)

    # SPMD pass sizing: max across cores, padded to 128
    pass_sizes = []
    for ph in range(NPH):
        K = max(len(per_core[i][ph]["passes"]) for i in range(N_CORES))
        sizes = []
        for k in range(K):
            L = max(
                len(per_core[i][ph]["passes"][k])
                if k < len(per_core[i][ph]["passes"])
                else 0
                for i in range(N_CORES)
            )
            sizes.append(-(-L // P) * P)
        pass_sizes.append(sizes)
    plan.pass_sizes = pass_sizes

    # real (unpadded) max-across-cores length of each pass
    real_sizes = []
    for ph in range(NPH):
        K = len(pass_sizes[ph])
        real_sizes.append(
            [
                max(
                    len(per_core[i][ph]["passes"][k])
                    if k < len(per_core[i][ph]["passes"])
                    else 0
                    for i in range(N_CORES)
                )
                for k in range(K)
            ]
        )
    plan.real_sizes = real_sizes

    # fused gather grouping (never split a pass); reg = slots minus the
    # trailing pad of the group's last pass (trimmed by num_idxs_reg)
    cap = max(FUSE_CAP, max((max(s) for s in pass_sizes if s), default=P))
    fuse = []
    for ph in range(NPH):
        groups = []
        k0, tot = 0, 0
        for k, n in enumerate(pass_sizes[ph]):
            if tot and tot + n > cap:
                last = k - 1
                groups.append(
                    (k0, k, tot, tot - pass_sizes[ph][last] + real_sizes[ph][last])
                )
                k0, tot = k, 0
            tot += n
        if tot:
            last = len(pass_sizes[ph]) - 1
            groups.append(
                (
                    k0,
                    len(pass_sizes[ph]),
                    tot,
                    tot - pass_sizes[ph][last] + real_sizes[ph][last],
                )
            )
        fuse.append(groups)
    plan.fuse = fuse

    g16_off, g128_off = [], []
    o16 = o128 = 0
    for ph in range(NPH):
        offs16, offs128 = [], []
        for n in pass_sizes[ph]:
            offs16.append(o16)
            offs128.append(o128)
            o16 += n // 16
            o128 += n // P
        g16_off.append(offs16)
        g128_off.append(offs128)
    plan.g16_off, plan.g128_off = g16_off, g128_off
    plan.GCOLS = max(o16, 16)
    plan.WCOLS = max(o128, 1)

    W = np.ascontiguousarray(np.asarray(W, dtype=np.float32))
    b = np.ascontiguousarray(np.asarray(b, dtype=np.float32).reshape(DIM, 1))
    MGCOLS = plan.mg_off(plan.n_mg - 1, N_CHUNKS - 1) + plan.mg_sizes[-1] // 16

    in_maps = []
    for i in range(N_CORES):
        gidx = np.zeros((P, plan.GCOLS), dtype=np.int16)
        gwgt = np.zeros((P, plan.WCOLS), dtype=np.float32)
        for ph in range(NPH):
            c = ph // N_QUART
            pdata = per_core[i][ph]
            for k, n in enumerate(pass_sizes[ph]):
                s_pass = (
                    pdata["passes"][k]
                    if k < len(pdata["passes"])
                    else np.empty(0, np.int64)
                )
                L = len(s_pass)
                iv = np.zeros(n, dtype=np.int16)
                wv = np.zeros(n, dtype=np.float32)
                iv[:L] = (s_pass - c * CH).astype(np.int16)
                wv[:L] = dinv[s_pass]
                gidx[:, g16_off[ph][k] : g16_off[ph][k] + n // 16] = _rep16(iv, n)
                gwgt[:, g128_off[ph][k] : g128_off[ph][k] + n // P] = (
                    wv.reshape(n // P, P).T
                )
        # merge idxs: accd[c] row for dst d = p*JJ + q*ACCB + j  (slot=j*128+p)
        # degree-0 (d,c) or pad dst -> zero row (col-block JJ-1, p=0)
        mgidx = np.zeros((P, MGCOLS), dtype=np.int16)
        zrow = plan.JJ - 1
        for mg in range(plan.n_mg):
            gsz = plan.mg_sizes[mg]
            d = np.arange(mg * MG_BLK * P, mg * MG_BLK * P + gsz)
            for c in range(N_CHUNKS):
                ivc = np.full(gsz, zrow, dtype=np.int32)
                valid = d < SHARD
                dv = d[valid]
                qv = dv // QH
                lv = dv - qv * QH
                for q in range(N_QUART):
                    ph = c * N_QUART + q
                    rank = per_core[i][ph]["rank"]
                    degq = per_core[i][ph]["deg"]
                    mq = qv == q
                    lq = lv[mq]
                    r = rank[lq]
                    out_idx = (r % P) * plan.JJ + q * plan.ACCB + r // P
                    out_idx = np.where(degq[lq] > 0, out_idx, zrow)
                    tmp = ivc[valid]
                    tmp[mq] = out_idx
                    ivc[valid] = tmp
                assert ivc.max() <= 32767
                o = plan.mg_off(mg, c)
                mgidx[:, o : o + gsz // 16] = _rep16(ivc.astype(np.int16), gsz)
        # dinv of dst, raster layout [p, block]
        dvv = np.zeros((plan.SHB * P,), dtype=np.float32)
        dvv[:SHARD] = dinv[i * SHARD : (i + 1) * SHARD]
        dinvd = np.ascontiguousarray(dvv.reshape(plan.SHB, P).T)
        # x shard pre-swizzled partition-major: [p, block*DIM]
        xpm = np.zeros((plan.SHB * P, DIM), dtype=np.float32)
        xpm[:SHARD] = x[i * SHARD : (i + 1) * SHARD]
        xsh_pm = np.ascontiguousarray(
            xpm.reshape(plan.SHB, P, DIM).transpose(1, 0, 2).reshape(P, plan.SHB * DIM)
        )
        in_maps.append(
            {
                "x": x,
                "xsh_pm": xsh_pm,
                "w": W,
                "bias": b,
                "dinvd": dinvd,
                "gidx": gidx,
                "gwgt": gwgt,
                "mgidx": mgidx,
            }
        )
    return plan, in_maps


_QPATCHED = [False]


def _patch_queue_aware_dma_lanes():
    """Partition the 8 DMASW completion-sem lanes so SWDGE queue q owns
    lanes {2q, 2q+1} (Tile's round-robin ignores queue_num; queues sharing a
    lane can complete out of order and release waiters early)."""
    if _QPATCHED[0]:
        return
    _QPATCHED[0] = True
    from concourse import tile_sem_assignment as tsa
    from concourse import bass_isa, mybir

    orig = tsa.TileClockTick._assign_tick

    def qaware(self, inst):
        if (
            isinstance(inst, tsa.DMAInst)
            and inst.engine == mybir.EngineType.Pool
            and not isinstance(inst, bass_isa.UserSyncedRemoteDMADescs)
        ):
            qn = getattr(inst, "queue_num", 0) or 0
            tog = getattr(self, "_q_toggle", None)
            if tog is None:
                tog = self._q_toggle = {}
            t = tog.get(qn, 0)
            tog[qn] = t ^ 1
            self.next_sw_dma_idx = 2 * qn + t
        return orig(self, inst)

    tsa.TileClockTick._assign_tick = qaware


def build_program(plan):
    from concourse import bacc, mybir
    import concourse.tile as tile
    from concourse.masks import make_identity
    from concourse.tile import add_dep_helper

    _patch_queue_aware_dma_lanes()

    N = plan.N
    SHARD, CH = plan.SHARD, plan.CH
    ACCB, ACC_COLS, JJ, SHB = plan.ACCB, plan.ACC_COLS, plan.JJ, plan.SHB
    NPH = plan.n_phases
    f32 = mybir.dt.float32
    bf16 = mybir.dt.bfloat16
    i16 = mybir.dt.int16
    mult = mybir.AluOpType.mult
    add = mybir.AluOpType.add
    Copy = mybir.ActivationFunctionType.Copy
    Relu = mybir.ActivationFunctionType.Relu

    nc = bacc.Bacc("TRN2", target_bir_lowering=False, num_swdge_queues=N_QUEUES)
    x_d = nc.dram_tensor("x", [N, DIM], f32, kind="ExternalInput")
    xsh_d = nc.dram_tensor("xsh_pm", [P, SHB * DIM], f32, kind="ExternalInput")
    w_d = nc.dram_tensor("w", [DIM, DIM], f32, kind="ExternalInput")
    b_d = nc.dram_tensor("bias", [DIM, 1], f32, kind="ExternalInput")
    dinvd_d = nc.dram_tensor("dinvd", [P, SHB], f32, kind="ExternalInput")
    gidx_d = nc.dram_tensor("gidx", [P, plan.GCOLS], i16, kind="ExternalInput")
    gwgt_d = nc.dram_tensor("gwgt", [P, plan.WCOLS], f32, kind="ExternalInput")
    MGCOLS = plan.mg_off(plan.n_mg - 1, N_CHUNKS - 1) + plan.mg_sizes[-1] // 16
    mgidx_d = nc.dram_tensor("mgidx", [P, MGCOLS], i16, kind="ExternalInput")
    accd = [
        nc.dram_tensor(f"accd{c}", [P * JJ, DIM], f32) for c in range(N_CHUNKS)
    ]
    agd = nc.dram_tensor("agd", [P, SHB * DIM], f32)
    out_d = nc.dram_tensor("out", [P, SHB * DIM], f32, kind="ExternalOutput")

    qload = [0.0] * N_QUEUES

    def next_q(cost):
        q = min(range(N_QUEUES), key=lambda i: qload[i])
        qload[q] += cost
        return q

    # per (phase, bank): last pass touching the bank (for matmul stop flags)
    def cols_of(nslots):
        return (nslots // P) * DIM

    last_pass = []
    banks_used = []
    for ph in range(NPH):
        sizes = plan.pass_sizes[ph]
        nb = -(-cols_of(sizes[0]) // BANK_F32) if sizes else 0
        banks_used.append(nb)
        lp = [0] * nb
        for bnk in range(nb):
            for k, n in enumerate(sizes):
                if cols_of(n) > bnk * BANK_F32:
                    lp[bnk] = k
        last_pass.append(lp)

    with tile.TileContext(nc) as tc:
        with (
            tc.tile_pool(name="const", bufs=1) as constp,
            tc.tile_pool(name="gbuf", bufs=10) as gbufp,
            tc.tile_pool(name="stg", bufs=2) as stgp,
            tc.tile_pool(name="mgt", bufs=2) as mgtp,
            tc.tile_pool(name="ep", bufs=2) as epp,
            tc.tile_pool(name="acc", bufs=1, space="PSUM") as accpsum,
            tc.tile_pool(name="ptz", bufs=1, space="PSUM") as ptzp,
            tc.tile_pool(name="pop", bufs=2, space="PSUM") as popp,
        ):
            ident = constp.tile([P, P], f32)
            make_identity(nc, ident[:])
            ident_bf = constp.tile([P, P], bf16)
            make_identity(nc, ident_bf[:])
            w_t = constp.tile([DIM, DIM], f32)
            nc.sync.dma_start(out=w_t[:], in_=w_d[:])
            w_bf = constp.tile([DIM, DIM], bf16)
            nc.vector.tensor_copy(out=w_bf[:], in_=w_t[:])
            b_t = constp.tile([DIM, 1], f32)
            nc.sync.dma_start(out=b_t[:], in_=b_d[:])
            dinvd_t = constp.tile([P, SHB], f32)
            nc.sync.dma_start(out=dinvd_t[:], in_=dinvd_d[:])
            mgidx_t = constp.tile([P, MGCOLS], i16)
            nc.sync.dma_start(out=mgidx_t[:], in_=mgidx_d[:])
            gidx_t = constp.tile([P, plan.GCOLS], i16)
            nc.sync.dma_start(out=gidx_t[:], in_=gidx_d[:])
            gwgt_t = constp.tile([P, plan.WCOLS], f32)
            nc.sync.dma_start(out=gwgt_t[:], in_=gwgt_d[:])
            zero_t = constp.tile([P, DIM], f32)
            nc.vector.memset(zero_t[:], 0.0)

            # one-time scrub of the gather-buffer pool: rows skipped by
            # num_idxs_reg trimming read stale SBUF; 0-weight pads would
            # propagate NaN bit patterns from uninitialized memory
            for _ in range(9):
                gscrub = gbufp.tile([P, (FUSE_CAP // P) * DIM], f32, tag="gb",
                                    name="gscrub", bufs=9)
                nc.vector.memset(gscrub[:], 0.0)

            # zero row of each accd table (degree-0 / padding reads)
            zwrites = []
            for c in range(N_CHUNKS):
                zw = nc.sync.dma_start(
                    out=accd[c]
                    .rearrange("(p jj) d -> p jj d", jj=JJ)[:, JJ - 1, :],
                    in_=zero_t[:],
                )
                zwrites.append(zw)

            # quarter emission order chosen so the last quarter gates the
            # fewest merge groups; a group is ready at the latest emission
            # position among the quarters it spans
            QORDER = [0, 1, 2, 3]
            pos_of_q = {q: i for i, q in enumerate(QORDER)}
            mg_ready = {i: [] for i in range(N_QUART)}
            for mg in range(plan.n_mg):
                mg_ready[max(pos_of_q[q] for q in plan.mg_quarters[mg])].append(
                    mg
                )

            # ---------------- phases: gather + PE accumulate -------------
            stg_writes = {}  # (c, q) -> dma inst
            phase_order = [(c, q) for q in QORDER for c in range(N_CHUNKS)]

            def emit_phase(c, q):
                ph = c * N_QUART + q
                sizes = plan.pass_sizes[ph]
                if not sizes:
                    return
                nbanks = banks_used[ph]
                accb = [
                    accpsum.tile([P, BANK_F32], f32, tag=f"accb{bnk}", name=f"accb{bnk}")
                    for bnk in range(nbanks)
                ]
                for gi, (k0, k1, slots, reg) in enumerate(plan.fuse[ph]):
                    nblk = slots // P
                    buf = gbufp.tile([P, (FUSE_CAP // P) * DIM], f32, tag="gb",
                                     name="gb", bufs=9)
                    bufb = gbufp.tile([P, (FUSE_CAP // P) * DIM], bf16, tag="gbb",
                                      name="bufb", bufs=4)
                    o16 = plan.g16_off[ph][k0]
                    o128 = plan.g128_off[ph][k0]
                    nc.gpsimd.dma_gather(
                        out_ap=buf[:, : nblk * DIM].rearrange(
                            "p (j d) -> p j d", d=DIM
                        ),
                        in_ap=x_d[c * CH : (c + 1) * CH, :],
                        idxs_ap=gidx_t[:, o16 : o16 + slots // 16],
                        num_idxs=slots,
                        num_idxs_reg=reg,
                        elem_size=DIM,
                        single_packet=False,
                        queue_num=next_q(1000 + 8 * reg),
                    )
                    nc.vector.tensor_tensor(
                        out=bufb[:, : nblk * DIM].rearrange("p (j d) -> p j d", d=DIM),
                        in0=buf[:, : nblk * DIM].rearrange("p (j d) -> p j d", d=DIM),
                        in1=gwgt_t[:, o128 : o128 + nblk].to_broadcast(
                            [P, nblk, DIM]
                        ),
                        op=mult,
                    )
                    boff = 0
                    for k in range(k0, k1):
                        ck = cols_of(sizes[k])
                        for bnk in range(-(-ck // BANK_F32)):
                            w0 = bnk * BANK_F32
                            w1 = min(w0 + BANK_F32, ck)
                            nc.tensor.matmul(
                                out=accb[bnk][:, : w1 - w0],
                                lhsT=ident_bf[:],
                                rhs=bufb[:, boff * DIM + w0 : boff * DIM + w1],
                                start=(k == 0),
                                stop=(k == last_pass[ph][bnk]),
                            )
                        boff += sizes[k] // P
                # drain psum banks -> staging, write accd
                stg = stgp.tile([P, ACC_COLS], f32, tag="stg", name="stg")
                c0 = cols_of(sizes[0])
                for bnk in range(nbanks):
                    w0 = bnk * BANK_F32
                    w1 = min(w0 + BANK_F32, c0)
                    nc.scalar.activation(
                        out=stg[:, w0:w1],
                        in_=accb[bnk][:, : w1 - w0],
                        func=Copy,
                    )
                sw = nc.sync.dma_start(
                    out=accd[c]
                    .rearrange("(p jj) d -> p jj d", jj=JJ)[
                        :, q * ACCB : (q + 1) * ACCB, :
                    ],
                    in_=stg[:].rearrange("p (j d) -> p j d", d=DIM),
                )
                stg_writes[(c, q)] = sw

            # ---------------- merge + epilogue ---------------------------
            def emit_merge_gathers(mg):
                gsz = plan.mg_sizes[mg]
                blks = gsz // P
                mgt = []
                for c in range(N_CHUNKS):
                    mb = mgtp.tile([P, MG_BLK * DIM], f32, tag=f"mg{c}",
                                   name=f"mg{c}")
                    o = plan.mg_off(mg, c)
                    ginst = nc.gpsimd.dma_gather(
                        out_ap=mb[:, : blks * DIM].rearrange(
                            "p (j d) -> p j d", d=DIM
                        ),
                        in_ap=accd[c][:, :],
                        idxs_ap=mgidx_t[:, o : o + gsz // 16],
                        num_idxs=gsz,
                        num_idxs_reg=gsz,
                        elem_size=DIM,
                        single_packet=False,
                        queue_num=next_q(1000 + 8 * gsz),
                    )
                    add_dep_helper(ginst.ins, zwrites[c].ins, reason="zero row")
                    for q in plan.mg_quarters[mg]:
                        add_dep_helper(
                            ginst.ins, stg_writes[(c, q)].ins,
                            reason="accd write before merge",
                        )
                    mgt.append(mb)
                return mgt

            def emit_merge_adds(mg, mgt):
                # agw = x*dinv + sum_c merged_c  (written to HBM; the tail
                # epilogue re-reads it).  Emitted >=2 phases after the merge
                # gathers so the in-order DVE stream never blocks on them.
                gsz = plan.mg_sizes[mg]
                cols = gsz // P * DIM
                base = mg * MG_BLK
                dv = dinvd_t[:, base : base + gsz // P].to_broadcast(
                    [P, gsz // P, DIM]
                )
                xg = mgtp.tile([P, MG_BLK * DIM], f32, tag="xgm", name="xgm")
                nc.sync.dma_start(
                    out=xg[:, :cols],
                    in_=xsh_d[:, base * DIM : base * DIM + cols],
                )
                agw = mgtp.tile([P, MG_BLK * DIM], f32, tag="agw", name="agw")
                nc.vector.tensor_tensor(
                    out=agw[:, :cols].rearrange("p (j d) -> p j d", d=DIM),
                    in0=xg[:, :cols].rearrange("p (j d) -> p j d", d=DIM),
                    in1=dv,
                    op=mult,
                )
                for c in range(N_CHUNKS):
                    nc.vector.tensor_tensor(
                        out=agw[:, :cols],
                        in0=agw[:, :cols],
                        in1=mgt[c][:, :cols],
                        op=add,
                    )
                return nc.sync.dma_start(
                    out=agd[:, base * DIM : base * DIM + cols],
                    in_=agw[:, :cols],
                )

            def emit_tail_epilogue(e, ag_writes):
                eb = plan.ep_sizes[e]
                cols = eb * DIM
                dv = dinvd_t[:, e * EP_BLK : e * EP_BLK + eb].to_broadcast(
                    [P, eb, DIM]
                )
                xg = epp.tile([P, EP_BLK * DIM], f32, tag="xg", name="xg")
                nc.sync.dma_start(
                    out=xg[:, :cols],
                    in_=xsh_d[:, e * EP_BLK * DIM : e * EP_BLK * DIM + cols],
                )
                agl = epp.tile([P, EP_BLK * DIM], f32, tag="agl", name="agl")
                ld = nc.sync.dma_start(
                    out=agl[:, :cols],
                    in_=agd[:, e * EP_BLK * DIM : e * EP_BLK * DIM + cols],
                )
                mg = (e * EP_BLK) // MG_BLK
                add_dep_helper(ld.ins, ag_writes[mg].ins, reason="agd before read")
                ag = epp.tile([P, EP_BLK * DIM], f32, tag="ag", name="ag")
                nc.vector.tensor_tensor(
                    out=ag[:, :cols].rearrange("p (j d) -> p j d", d=DIM),
                    in0=agl[:, :cols].rearrange("p (j d) -> p j d", d=DIM),
                    in1=dv,
                    op=mult,
                )
                pt = ptzp.tile([DIM, EP_BLK * P], f32, tag="ptz", name="pt")
                for bb in range(eb):
                    nc.tensor.transpose(
                        out=pt[:, bb * P : (bb + 1) * P],
                        in_=ag[:, bb * DIM : (bb + 1) * DIM],
                        identity=ident[:],
                    )
                at = epp.tile([DIM, EP_BLK * P], bf16, tag="at", name="at")
                nc.scalar.activation(
                    out=at[:, : eb * P], in_=pt[:, : eb * P], func=Copy
                )
                pz = ptzp.tile([DIM, EP_BLK * P], f32, tag="ptz", name="pz")
                nc.tensor.matmul(
                    out=pz[:, : eb * P],
                    lhsT=w_bf[:],
                    rhs=at[:, : eb * P],
                    start=True,
                    stop=True,
                )
                zr = epp.tile([DIM, EP_BLK * P], f32, tag="zr", name="zr")
                nc.scalar.activation(
                    out=zr[:, : eb * P],
                    in_=pz[:, : eb * P],
                    func=Relu,
                    bias=b_t[:],
                )
                po = popp.tile([P, EP_BLK * DIM], f32, tag="po", name="po")
                for bb in range(eb):
                    nc.tensor.transpose(
                        out=po[:, bb * DIM : (bb + 1) * DIM],
                        in_=zr[:, bb * P : (bb + 1) * P],
                        identity=ident[:DIM, :DIM],
                    )
                ot = epp.tile([P, EP_BLK * DIM], f32, tag="ot", name="ot")
                nc.vector.tensor_tensor(
                    out=ot[:, :cols],
                    in0=po[:, :cols],
                    in1=xg[:, :cols],
                    op=add,
                )
                nc.sync.dma_start(
                    out=out_d[:, e * EP_BLK * DIM : e * EP_BLK * DIM + cols],
                    in_=ot[:, :cols],
                )

            # driver: quarter-major phases.  A quarter's merge gathers are
            # emitted as soon as it completes (early SWDGE queue position);
            # their DVE adds two phases later (data ready, so the in-order
            # DVE stream never blocks); each group's epilogues two phases
            # after that.
            pend_g = []     # [mg, phases_remaining] merge gathers to emit
            pend_adds = []  # [mg, mgt, phases_remaining]
            pend_eps = []   # [ep_group, phases_remaining]
            eps_emitted = set()
            ag_writes = {}

            def queue_eps(mg):
                e0 = mg * (MG_BLK // EP_BLK)
                for e in range(e0, min(e0 + MG_BLK // EP_BLK, plan.n_ep)):
                    pend_eps.append([e, 1])

            def tick_pending(force=False):
                # merge gathers one phase after their quarter: the accd
                # writes they wait on have landed, so they never park at
                # the head of the in-order Pool stream
                for item in list(pend_g):
                    mg, rem = item
                    if rem <= 0 or force:
                        pend_adds.append([mg, emit_merge_gathers(mg), 2])
                        pend_g.remove(item)
                    else:
                        item[1] = rem - 1
                for item in list(pend_adds):
                    mg, mgt, rem = item
                    if rem <= 0 or force:
                        ag_writes[mg] = emit_merge_adds(mg, mgt)
                        pend_adds.remove(item)
                        queue_eps(mg)
                    else:
                        item[2] = rem - 1
                # drain at most 2 epilogue groups per tick: their only DVE
                # op depends on loads (agd written a tick earlier), and the
                # cap keeps the in-order DVE/PE streams from bunching
                n = 0
                for item in list(pend_eps):
                    e, rem = item
                    if rem <= 0 and (n < 2 or force):
                        emit_tail_epilogue(e, ag_writes)
                        eps_emitted.add(e)
                        pend_eps.remove(item)
                        n += 1
                    else:
                        item[1] = rem - 1

            for pi, (c, q) in enumerate(phase_order):
                emit_phase(c, q)
                tick_pending()
                if c == N_CHUNKS - 1:
                    pos = pos_of_q[q]
                    for mg in mg_ready[pos]:
                        if pos == N_QUART - 1:
                            # tail: adds inline so mgt pool rotation never
                            # needs a dep on a not-yet-emitted consumer
                            tick_pending(force=True)
                            ag_writes[mg] = emit_merge_adds(
                                mg, emit_merge_gathers(mg)
                            )
                            queue_eps(mg)
                        else:
                            pend_g.append([mg, 1])
            tick_pending(force=True)
            for e in range(plan.n_ep):
                if e not in eps_emitted:
                    emit_tail_epilogue(e, ag_writes)

    nc.compile()
    return nc


def _install_ntff_hook():
    if "antenv.axon_hooks" in sys.modules:
        return
    try:
        sys.path.insert(0, "/root/.axon_site")
        from trn_agent_boot.trn_boot import _ntff_profile_via_ctypes

        hook = _ntff_profile_via_ctypes("/opt/axon/libaxon_pjrt.so")
    except Exception:
        hook = None
    mod = types.ModuleType("antenv.axon_hooks")
    mod.get_axon_ntff_profile_hook = lambda: hook
    mod.set_axon_ntff_profile_hook = lambda h: None
    sys.modules["antenv.axon_hooks"] = mod


def run(plan, nc, in_maps, trace=False, tmpdir=None):
    _install_ntff_hook()
    from concourse.bass_utils import run_bass_kernel_spmd

    res = run_bass_kernel_spmd(
        nc,
        in_maps,
        core_ids=list(range(N_CORES)),
        trace=trace,
        tmpdir=tmpdir,
    )
    outs = []
    for i in range(N_CORES):
        raw = res.results[i]["out"]  # [P, SHB*DIM] partition-major
        full = (
            raw.reshape(P, plan.SHB, DIM)
            .transpose(1, 0, 2)
            .reshape(plan.SHB * P, DIM)[: plan.SHARD]
        )
        outs.append(full)
    return np.concatenate(outs, axis=0), res


_CACHE = {}


def kernel(x, edge_index, W, b):
    plan, in_maps = preprocess(x, edge_index, W, b)
    sig = tuple(tuple(s) for s in plan.pass_sizes)
    ent = _CACHE.get("prog")
    if ent is None or ent[0] != sig:
        nc = build_program(plan)
        _CACHE["prog"] = (sig, nc)
    nc = _CACHE["prog"][1]
    out, _ = run(plan, nc, in_maps)
    return out
